# revision 10
# baseline (speedup 1.0000x reference)
"""Pointer-attention kernel for Trainium2 (8 NeuronCores, data-parallel over batch).

Computes, for P = pointer_input [B, S, R], weights W1/W2 [2R]:
    scores = P @ W1[:R] + (h @ W1[R:])[:, None]      # h-term is constant over S
    a      = softmax(scores, axis=S)                 #   -> cancels in softmax
    c      = einsum('bsr,bs->br', P, a)
    pi     = P @ W2[:R] + (c @ W2[R:])[:, None]

Math used here (exact):
    s1[b,s]  = P[b,s,:] . w1p          (w1p = W1[:R])
    E        = exp(s1)                 (softmax shift cancels; inputs are O(1))
    Z[b]     = sum_s E[b,s]
    craw[b,:]= sum_s E[b,s] * P[b,s,:]
    g[b]     = (craw[b,:] . w2c) / Z[b]            (w2c = W2[R:])
    pi[b,s]  = P[b,s,:] . w2p + g[b]               (w2p = W2[:R])

so h_t and W1[R:] never affect the output. One single pass over P.

Measured-cost engine split (per 128x512 s-tile; 8 batches x 16 tiles per core):
  - input DMA on HWDGE (nc.sync), 1 MiB super-tiles (4 tiles) - Pool does no DMA
  - craw on TensorE: lhsT = exp column (bf16), rhs = the *high half-word view*
    of the fp32 tile (stride-2 bf16 bitcast) -> no bf16 cast of P ever happens
  - the 256 matvec tasks (s1, pw2 per tile) split three ways by measured cost:
      a-path: DVE fused scalar_tensor_tensor fp32 (~0.9us)
      q-path: Pool tensor_mul fp32 super-tile (~0.9us/tile) + ScalarE
              activation-accumulate reduce (~1.0us)
      p-path: Pool mult + DVE flat tensor_reduce (~0.67us)
    balanced so DVE/Pool/ACT all land just under the ~94us/core DMA roofline.
  - exp on ScalarE per super-tile, written directly as bf16
Per-b epilogue: Z via ones-matmul, dq = craw.w2c (fused DVE op), g = dq/Z,
broadcast via ones-matmul, pi = pw2 + g on ScalarE, DMA out on HWDGE.
"""

import numpy as np

B, S, R = 64, 2048, 512
N_CORES = 8
B_LOC = B // N_CORES          # 8 batches per core
P_PART = 128                  # partitions per s-tile
NT = S // P_PART              # 16 s-tiles per batch

_CACHED_NC = None

# build-time strategy knobs (tuned from traces)
CFG = dict(
    st=8,            # s-tiles per DMA super-tile (8 -> 2 MiB transfers)
    pool_chunks=23,  # of 64 (half-super-tile x weight) mult-chunks on Pool
    dve_red_5th=0,   # every Nth pool reduce-task on DVE (0 = all on ACT)
    dma_eng="sync",  # sync | gpsimd
)


def _build_nc(cfg=None, b_loc=B_LOC, nt=NT, finalize=True):
    import concourse.bacc as bacc
    import concourse.bass as bass
    import concourse.mybir as mybir
    import concourse.tile as tile

    cfg = dict(CFG, **(cfg or {}))
    f32 = mybir.dt.float32
    bf16 = mybir.dt.bfloat16
    st_sz = cfg["st"]
    s_loc = nt * P_PART
    assert nt % st_sz == 0
    nst = nt // st_sz
    # mult-chunk = (super-tile, weight, half): 4 s-tiles each, 64 per core
    n_chunks = b_loc * nst * 2 * 2
    pool_set = set()
    acc = 0
    for c in range(n_chunks):
        acc += cfg["pool_chunks"]
        if acc >= n_chunks:
            acc -= n_chunks
            pool_set.add(c)
    nc = bacc.Bacc(None, target_bir_lowering=False, debug=True)

    p_h = nc.declare_dram_parameter("p", [b_loc, s_loc, R], f32, isOutput=False)
    w1_h = nc.declare_dram_parameter("w1", [2 * R], f32, isOutput=False)
    w2_h = nc.declare_dram_parameter("w2", [2 * R], f32, isOutput=False)
    out_h = nc.declare_dram_parameter("out", [b_loc, s_loc], f32, isOutput=True)

    def bcast_ap(src_ap, parts, rep=1):
        # replicate a 1-D DRAM slice across `parts` partitions, `rep` times
        ap = [[0, parts]]
        if rep > 1:
            ap.append([0, rep])
        ap += [list(d) for d in src_ap.ap]
        return bass.AP(tensor=src_ap.tensor, offset=src_ap.offset, ap=ap)

    with tile.TileContext(nc) as tc:
        dma_eng = nc.sync if cfg["dma_eng"] == "sync" else nc.gpsimd
        with (
            tc.tile_pool(name="consts", bufs=1) as consts,
            tc.tile_pool(name="ptiles", bufs=6) as ptiles,
            tc.tile_pool(name="prods", bufs=4) as prods,
            tc.tile_pool(name="scratch", bufs=6) as scratch,
            tc.tile_pool(name="perb", bufs=3) as perb,
            tc.tile_pool(name="smalls", bufs=3) as smalls,
            tc.tile_pool(name="psum_c", bufs=2, space="PSUM") as psum_c,
            tc.tile_pool(name="psum_s", bufs=2, space="PSUM") as psum_s,
        ):
            # ---- constants ----
            w1p = consts.tile([P_PART, R], f32)
            nc.gpsimd.dma_start(out=w1p[:], in_=bcast_ap(w1_h[0:R], P_PART))
            w2p = consts.tile([P_PART, R], f32)
            nc.gpsimd.dma_start(out=w2p[:], in_=bcast_ap(w2_h[0:R], P_PART))
            # super-tile-wide repeats for the Pool mult path
            half_t = st_sz // 2
            w1r = consts.tile([P_PART, half_t * R], f32)
            nc.gpsimd.dma_start(
                out=w1r[:], in_=bcast_ap(w1_h[0:R], P_PART, rep=half_t)
            )
            w2r = consts.tile([P_PART, half_t * R], f32)
            nc.gpsimd.dma_start(
                out=w2r[:], in_=bcast_ap(w2_h[0:R], P_PART, rep=half_t)
            )
            w2c = consts.tile([1, R], f32)
            nc.gpsimd.dma_start(out=w2c[:], in_=bcast_ap(w2_h[R : 2 * R], 1))
            ones_col = consts.tile([P_PART, 1], f32)
            nc.vector.memset(ones_col[:], 1.0)
            ones_row = consts.tile([1, P_PART], f32)
            nc.vector.memset(ones_row[:], 1.0)

            pool_task = 0  # running index of pool-chunk reduce tasks
            for b in range(b_loc):
                c_ps = psum_c.tile([1, R], f32, tag="c_ps")
                s1_b = perb.tile([P_PART, nt], f32, tag="s1_b")
                pw2_b = perb.tile([P_PART, nt], f32, tag="pw2_b")
                e_bf = perb.tile([P_PART, nt], bf16, tag="e_bf")

                for sti in range(nst):
                    pt = ptiles.tile([P_PART, st_sz * R], f32, tag="pt")
                    src = p_h[b, sti * st_sz * P_PART : (sti + 1) * st_sz * P_PART, :]
                    # p-major: partition p holds st_sz consecutive DRAM rows
                    # -> 8 KB contiguous runs, 4x fewer DMA packets.
                    # Two 1 MiB halves: consumers of the first half start
                    # before the second half lands (half-granular deps).
                    src2 = src.rearrange("(p t) r -> p (t r)", p=P_PART)
                    hw = st_sz * R // 2
                    dma_eng.dma_start(out=pt[:, :hw], in_=src2[:, :hw])
                    dma_eng.dma_start(out=pt[:, hw:], in_=src2[:, hw:])

                    for w in range(2):
                        wr = w1r if w == 0 else w2r
                        wp = w1p if w == 0 else w2p
                        tgt = s1_b if w == 0 else pw2_b
                        for h in range(2):
                            chunk = (((b * nst + sti) * 2) + w) * 2 + h
                            j0 = h * half_t
                            if chunk in pool_set:
                                # q-path: Pool mult over half the super-tile
                                prodP = prods.tile(
                                    [P_PART, half_t * R], f32, tag="prodP"
                                )
                                nc.gpsimd.tensor_mul(
                                    prodP[:],
                                    pt[:, j0 * R : (j0 + half_t) * R],
                                    wr[:],
                                )
                                for j in range(half_t):
                                    t = sti * st_sz + j0 + j
                                    seg = prodP[:, j * R : (j + 1) * R]
                                    n = cfg["dve_red_5th"]
                                    if n and pool_task % n == 0:
                                        nc.vector.tensor_reduce(
                                            out=tgt[:, t : t + 1], in_=seg,
                                            axis=mybir.AxisListType.X,
                                            op=mybir.AluOpType.add,
                                        )
                                    else:
                                        dump = scratch.tile(
                                            [P_PART, R], bf16, tag="dump"
                                        )
                                        nc.scalar.activation(
                                            out=dump[:], in_=seg,
                                            func=mybir.ActivationFunctionType.Identity,
                                            bias=0.0, scale=1.0,
                                            accum_out=tgt[:, t : t + 1],
                                        )
                                    pool_task += 1
                            else:
                                # a-path: fused stt on DVE per tile
                                for j in range(half_t):
                                    t = sti * st_sz + j0 + j
                                    prod = scratch.tile(
                                        [P_PART, R], f32, tag="prod"
                                    )
                                    nc.vector.scalar_tensor_tensor(
                                        out=prod[:],
                                        in0=pt[:, (j0 + j) * R : (j0 + j + 1) * R],
                                        scalar=1.0,
                                        in1=wp[:],
                                        op0=mybir.AluOpType.mult,
                                        op1=mybir.AluOpType.mult,
                                        accum_out=tgt[:, t : t + 1],
                                    )

                    # ---- exp of this super-tile's score columns (bf16 out) ----
                    nc.scalar.activation(
                        out=e_bf[:, sti * st_sz : (sti + 1) * st_sz],
                        in_=s1_b[:, sti * st_sz : (sti + 1) * st_sz],
                        func=mybir.ActivationFunctionType.Exp,
                    )
                    # ---- craw accumulation on TensorE (hi-half bf16 view) ----
                    for j in range(st_sz):
                        t = sti * st_sz + j
                        rhs_hi = pt[:, j * R : (j + 1) * R].bitcast(bf16)[:, 1::2]
                        nc.tensor.matmul(
                            c_ps[:],
                            lhsT=e_bf[:, t : t + 1],
                            rhs=rhs_hi,
                            start=(t == 0),
                            stop=(t == nt - 1),
                        )

                # ---- per-batch epilogue (all tiny, fp32) ----
                es = smalls.tile([P_PART, 1], f32, tag="es")
                nc.vector.reduce_sum(es[:], e_bf[:], axis=mybir.AxisListType.X)
                z_ps = psum_s.tile([1, 1], f32, tag="z_ps")
                nc.tensor.matmul(
                    z_ps[:], lhsT=es[:], rhs=ones_col[:], start=True, stop=True
                )
                c_sb = smalls.tile([1, R], f32, tag="c_sb")
                nc.scalar.copy(out=c_sb[:], in_=c_ps[:])
                zr = smalls.tile([1, 1], f32, tag="zr")
                nc.vector.reciprocal(out=zr[:], in_=z_ps[:])
                cprod = smalls.tile([1, R], f32, tag="cprod")
                dq = smalls.tile([1, 1], f32, tag="dq")
                nc.vector.scalar_tensor_tensor(
                    out=cprod[:],
                    in0=c_sb[:],
                    scalar=1.0,
                    in1=w2c[:],
                    op0=mybir.AluOpType.mult,
                    op1=mybir.AluOpType.mult,
                    accum_out=dq[:],
                )
                g = smalls.tile([1, 1], f32, tag="g")
                nc.vector.tensor_mul(g[:], dq[:], zr[:])
                g_ps = psum_s.tile([P_PART, 1], f32, tag="g_ps")
                nc.tensor.matmul(
                    g_ps[:], lhsT=ones_row[:], rhs=g[:], start=True, stop=True
                )
                g_bc = smalls.tile([P_PART, 1], f32, tag="g_bc")
                nc.scalar.copy(out=g_bc[:], in_=g_ps[:])
                pi_b = perb.tile([P_PART, nt], f32, tag="pi_b")
                nc.scalar.activation(
                    out=pi_b[:],
                    in_=pw2_b[:],
                    func=mybir.ActivationFunctionType.Identity,
                    bias=g_bc[:],
                    scale=1.0,
                )
                # s decomposes as (st, p, j): s = st*(128*st_sz) + p*st_sz + j.
                # pi_b free index is (st, j); 16 B DRAM runs, on the scalar
                # HWDGE ring so the tiny packets never block the input stream.
                nc.scalar.dma_start(
                    out=out_h[b].rearrange(
                        "(st p j) -> p st j", p=P_PART, j=st_sz
                    ),
                    in_=pi_b[:],
                )

    if finalize:
        nc.finalize()
    return nc


def _get_nc():
    global _CACHED_NC
    if _CACHED_NC is None:
        _CACHED_NC = _build_nc()
    return _CACHED_NC


def run_sharded(pointer_input, W1, W2, trace=False, trace_kwargs=None, nc=None):
    """Run the SPMD kernel; returns (full_output [1,B,S], BassKernelResults)."""
    from concourse.bass_utils import run_bass_kernel_spmd

    if nc is None:
        nc = _get_nc()
    pointer_input = np.ascontiguousarray(pointer_input, dtype=np.float32)
    W1 = np.ascontiguousarray(W1, dtype=np.float32)
    W2 = np.ascontiguousarray(W2, dtype=np.float32)
    in_maps = [
        {
            "p": pointer_input[i * B_LOC : (i + 1) * B_LOC],
            "w1": W1,
            "w2": W2,
        }
        for i in range(N_CORES)
    ]
    kw = dict(trace_kwargs or {})
    res = run_bass_kernel_spmd(
        nc, in_maps, list(range(N_CORES)), trace=trace, **kw
    )
    out = np.concatenate([res.results[i]["out"] for i in range(N_CORES)], axis=0)
    return out[None].astype(np.float32), res


def kernel(pointer_input, h_t, W1, W2):
    # h_t only shifts scores by a per-batch constant, which softmax cancels;
    # it does not affect the output.
    out, _ = run_sharded(pointer_input, W1, W2, trace=False)
    return out


# revision 12
# speedup vs baseline: 1.0744x; 1.0744x over previous
"""Pointer-attention kernel for Trainium2 (8 NeuronCores, data-parallel over batch).

Computes, for P = pointer_input [B, S, R], weights W1/W2 [2R]:
    scores = P @ W1[:R] + (h @ W1[R:])[:, None]      # h-term is constant over S
    a      = softmax(scores, axis=S)                 #   -> cancels in softmax
    c      = einsum('bsr,bs->br', P, a)
    pi     = P @ W2[:R] + (c @ W2[R:])[:, None]

Math used here (exact):
    s1[b,s]  = P[b,s,:] . w1p          (w1p = W1[:R])
    E        = exp(s1)                 (softmax shift cancels; inputs are O(1))
    Z[b]     = sum_s E[b,s]
    craw[b,:]= sum_s E[b,s] * P[b,s,:]
    g[b]     = (craw[b,:] . w2c) / Z[b]            (w2c = W2[R:])
    pi[b,s]  = P[b,s,:] . w2p + g[b]               (w2p = W2[:R])

so h_t and W1[R:] never affect the output. One single pass over P.

Measured-cost engine split (per 128x512 s-tile; 8 batches x 16 tiles per core):
  - input DMA on HWDGE (nc.sync), 1 MiB super-tiles (4 tiles) - Pool does no DMA
  - craw on TensorE: lhsT = exp column (bf16), rhs = the *high half-word view*
    of the fp32 tile (stride-2 bf16 bitcast) -> no bf16 cast of P ever happens
  - the 256 matvec tasks (s1, pw2 per tile) split three ways by measured cost:
      a-path: DVE fused scalar_tensor_tensor fp32 (~0.9us)
      q-path: Pool tensor_mul fp32 super-tile (~0.9us/tile) + ScalarE
              activation-accumulate reduce (~1.0us)
      p-path: Pool mult + DVE flat tensor_reduce (~0.67us)
    balanced so DVE/Pool/ACT all land just under the ~94us/core DMA roofline.
  - exp on ScalarE per super-tile, written directly as bf16
Per-b epilogue: Z via ones-matmul, dq = craw.w2c (fused DVE op), g = dq/Z,
broadcast via ones-matmul, pi = pw2 + g on ScalarE, DMA out on HWDGE.
"""

import numpy as np

B, S, R = 64, 2048, 512
N_CORES = 8
B_LOC = B // N_CORES          # 8 batches per core
P_PART = 128                  # partitions per s-tile
NT = S // P_PART              # 16 s-tiles per batch

_CACHED_NC = None

# build-time strategy knobs (tuned from traces)
CFG = dict(
    st=8,            # s-tiles per DMA super-tile (8 -> 2 MiB transfers)
    pool_chunks=23,  # of 64 (half-super-tile x weight) mult-chunks on Pool
    dve_red_5th=0,   # every Nth pool reduce-task on DVE (0 = all on ACT)
    dma_eng="sync",  # sync | gpsimd
)


def _build_nc(cfg=None, b_loc=B_LOC, nt=NT, finalize=True):
    import concourse.bacc as bacc
    import concourse.bass as bass
    import concourse.mybir as mybir
    import concourse.tile as tile

    cfg = dict(CFG, **(cfg or {}))
    f32 = mybir.dt.float32
    bf16 = mybir.dt.bfloat16
    st_sz = cfg["st"]
    s_loc = nt * P_PART
    assert nt % st_sz == 0
    nst = nt // st_sz
    # mult-chunk = (super-tile, weight, half): 4 s-tiles each, 64 per core
    n_chunks = b_loc * nst * 2 * 2
    pool_set = set()
    acc = 0
    for c in range(n_chunks):
        acc += cfg["pool_chunks"]
        if acc >= n_chunks:
            acc -= n_chunks
            pool_set.add(c)
    nc = bacc.Bacc(None, target_bir_lowering=False, debug=True)

    p_h = nc.declare_dram_parameter("p", [b_loc, s_loc, R], f32, isOutput=False)
    w1_h = nc.declare_dram_parameter("w1", [2 * R], f32, isOutput=False)
    w2_h = nc.declare_dram_parameter("w2", [2 * R], f32, isOutput=False)
    out_h = nc.declare_dram_parameter("out", [b_loc, s_loc], f32, isOutput=True)

    def bcast_ap(src_ap, parts, rep=1):
        # replicate a 1-D DRAM slice across `parts` partitions, `rep` times
        ap = [[0, parts]]
        if rep > 1:
            ap.append([0, rep])
        ap += [list(d) for d in src_ap.ap]
        return bass.AP(tensor=src_ap.tensor, offset=src_ap.offset, ap=ap)

    with tile.TileContext(nc) as tc:
        dma_eng = nc.sync if cfg["dma_eng"] == "sync" else nc.gpsimd
        with (
            tc.tile_pool(name="consts", bufs=1) as consts,
            tc.tile_pool(name="ptiles", bufs=6) as ptiles,
            tc.tile_pool(name="prods", bufs=4) as prods,
            tc.tile_pool(name="scratch", bufs=6) as scratch,
            tc.tile_pool(name="perb", bufs=3) as perb,
            tc.tile_pool(name="smalls", bufs=3) as smalls,
            tc.tile_pool(name="psum_c", bufs=2, space="PSUM") as psum_c,
            tc.tile_pool(name="psum_s", bufs=2, space="PSUM") as psum_s,
        ):
            # ---- constants ----
            # Tiny row loads from DRAM; partition-broadcast built on-chip
            # (PE ones-matmul + DVE copies) instead of 2.6 MB of slow
            # stride-0 broadcast DMA that stalls the input stream at start.
            half_t = st_sz // 2
            w1row = consts.tile([1, R], f32)
            nc.sync.dma_start(out=w1row[:], in_=bcast_ap(w1_h[0:R], 1))
            w2row = consts.tile([1, R], f32)
            nc.sync.dma_start(out=w2row[:], in_=bcast_ap(w2_h[0:R], 1))
            w2c = consts.tile([1, R], f32)
            nc.sync.dma_start(
                out=w2c[:], in_=bcast_ap(w2_h[R : 2 * R], 1)
            )
            ones_col = consts.tile([P_PART, 1], f32)
            nc.vector.memset(ones_col[:], 1.0)
            ones_row = consts.tile([1, P_PART], f32)
            nc.vector.memset(ones_row[:], 1.0)
            w1r = consts.tile([P_PART, half_t * R], f32)
            w2r = consts.tile([P_PART, half_t * R], f32)
            with tc.tile_pool(name="wps", bufs=2, space="PSUM") as wps:
                for wrow, wr in ((w1row, w1r), (w2row, w2r)):
                    w_ps = wps.tile([P_PART, R], f32, tag="w_ps")
                    nc.tensor.matmul(
                        w_ps[:], lhsT=ones_row[:], rhs=wrow[:],
                        start=True, stop=True,
                    )
                    nc.scalar.copy(out=wr[:, 0:R], in_=w_ps[:])
                    for rep in range(1, half_t):
                        nc.vector.tensor_copy(
                            wr[:, rep * R : (rep + 1) * R], wr[:, 0:R]
                        )
            w1p = w1r[:, 0:R]
            w2p = w2r[:, 0:R]

            pool_task = 0  # running index of pool-chunk reduce tasks
            for b in range(b_loc):
                c_ps = psum_c.tile([1, R], f32, tag="c_ps")
                s1_b = perb.tile([P_PART, nt], f32, tag="s1_b")
                pw2_b = perb.tile([P_PART, nt], f32, tag="pw2_b")
                e_bf = perb.tile([P_PART, nt], bf16, tag="e_bf")

                for sti in range(nst):
                    pt = ptiles.tile([P_PART, st_sz * R], f32, tag="pt")
                    src = p_h[b, sti * st_sz * P_PART : (sti + 1) * st_sz * P_PART, :]
                    # p-major: partition p holds st_sz consecutive DRAM rows
                    # -> 8 KB contiguous runs, 4x fewer DMA packets.
                    # Two 1 MiB halves: consumers of the first half start
                    # before the second half lands (half-granular deps).
                    src2 = src.rearrange("(p t) r -> p (t r)", p=P_PART)
                    hw = st_sz * R // 2
                    dma_eng.dma_start(out=pt[:, :hw], in_=src2[:, :hw])
                    dma_eng.dma_start(out=pt[:, hw:], in_=src2[:, hw:])

                    for w in range(2):
                        wr = w1r if w == 0 else w2r
                        wp = w1p if w == 0 else w2p  # AP views into w?r
                        tgt = s1_b if w == 0 else pw2_b
                        for h in range(2):
                            chunk = (((b * nst + sti) * 2) + w) * 2 + h
                            j0 = h * half_t
                            if chunk in pool_set:
                                # q-path: Pool mult over half the super-tile
                                prodP = prods.tile(
                                    [P_PART, half_t * R], f32, tag="prodP"
                                )
                                nc.gpsimd.tensor_mul(
                                    prodP[:],
                                    pt[:, j0 * R : (j0 + half_t) * R],
                                    wr[:],
                                )
                                for j in range(half_t):
                                    t = sti * st_sz + j0 + j
                                    seg = prodP[:, j * R : (j + 1) * R]
                                    n = cfg["dve_red_5th"]
                                    if n and pool_task % n == 0:
                                        nc.vector.tensor_reduce(
                                            out=tgt[:, t : t + 1], in_=seg,
                                            axis=mybir.AxisListType.X,
                                            op=mybir.AluOpType.add,
                                        )
                                    else:
                                        dump = scratch.tile(
                                            [P_PART, R], bf16, tag="dump"
                                        )
                                        nc.scalar.activation(
                                            out=dump[:], in_=seg,
                                            func=mybir.ActivationFunctionType.Identity,
                                            bias=0.0, scale=1.0,
                                            accum_out=tgt[:, t : t + 1],
                                        )
                                    pool_task += 1
                            else:
                                # a-path: fused stt on DVE per tile
                                for j in range(half_t):
                                    t = sti * st_sz + j0 + j
                                    prod = scratch.tile(
                                        [P_PART, R], f32, tag="prod"
                                    )
                                    nc.vector.scalar_tensor_tensor(
                                        out=prod[:],
                                        in0=pt[:, (j0 + j) * R : (j0 + j + 1) * R],
                                        scalar=1.0,
                                        in1=wp,
                                        op0=mybir.AluOpType.mult,
                                        op1=mybir.AluOpType.mult,
                                        accum_out=tgt[:, t : t + 1],
                                    )

                    # ---- exp of this super-tile's score columns (bf16 out) ----
                    nc.scalar.activation(
                        out=e_bf[:, sti * st_sz : (sti + 1) * st_sz],
                        in_=s1_b[:, sti * st_sz : (sti + 1) * st_sz],
                        func=mybir.ActivationFunctionType.Exp,
                    )
                    # ---- craw accumulation on TensorE (hi-half bf16 view) ----
                    for j in range(st_sz):
                        t = sti * st_sz + j
                        rhs_hi = pt[:, j * R : (j + 1) * R].bitcast(bf16)[:, 1::2]
                        nc.tensor.matmul(
                            c_ps[:],
                            lhsT=e_bf[:, t : t + 1],
                            rhs=rhs_hi,
                            start=(t == 0),
                            stop=(t == nt - 1),
                        )

                # ---- per-batch epilogue (all tiny, fp32) ----
                es = smalls.tile([P_PART, 1], f32, tag="es")
                nc.vector.reduce_sum(es[:], e_bf[:], axis=mybir.AxisListType.X)
                z_ps = psum_s.tile([1, 1], f32, tag="z_ps")
                nc.tensor.matmul(
                    z_ps[:], lhsT=es[:], rhs=ones_col[:], start=True, stop=True
                )
                c_sb = smalls.tile([1, R], f32, tag="c_sb")
                nc.scalar.copy(out=c_sb[:], in_=c_ps[:])
                zr = smalls.tile([1, 1], f32, tag="zr")
                nc.vector.reciprocal(out=zr[:], in_=z_ps[:])
                cprod = smalls.tile([1, R], f32, tag="cprod")
                dq = smalls.tile([1, 1], f32, tag="dq")
                nc.vector.scalar_tensor_tensor(
                    out=cprod[:],
                    in0=c_sb[:],
                    scalar=1.0,
                    in1=w2c[:],
                    op0=mybir.AluOpType.mult,
                    op1=mybir.AluOpType.mult,
                    accum_out=dq[:],
                )
                g = smalls.tile([1, 1], f32, tag="g")
                nc.vector.tensor_mul(g[:], dq[:], zr[:])
                g_ps = psum_s.tile([P_PART, 1], f32, tag="g_ps")
                nc.tensor.matmul(
                    g_ps[:], lhsT=ones_row[:], rhs=g[:], start=True, stop=True
                )
                g_bc = smalls.tile([P_PART, 1], f32, tag="g_bc")
                nc.scalar.copy(out=g_bc[:], in_=g_ps[:])
                pi_b = perb.tile([P_PART, nt], f32, tag="pi_b")
                nc.scalar.activation(
                    out=pi_b[:],
                    in_=pw2_b[:],
                    func=mybir.ActivationFunctionType.Identity,
                    bias=g_bc[:],
                    scale=1.0,
                )
                # s decomposes as (st, p, j): s = st*(128*st_sz) + p*st_sz + j.
                # pi_b free index is (st, j); 16 B DRAM runs, on the scalar
                # HWDGE ring so the tiny packets never block the input stream.
                nc.scalar.dma_start(
                    out=out_h[b].rearrange(
                        "(st p j) -> p st j", p=P_PART, j=st_sz
                    ),
                    in_=pi_b[:],
                )

    if finalize:
        nc.finalize()
    return nc


def _get_nc():
    global _CACHED_NC
    if _CACHED_NC is None:
        _CACHED_NC = _build_nc()
    return _CACHED_NC


def run_sharded(pointer_input, W1, W2, trace=False, trace_kwargs=None, nc=None):
    """Run the SPMD kernel; returns (full_output [1,B,S], BassKernelResults)."""
    from concourse.bass_utils import run_bass_kernel_spmd

    if nc is None:
        nc = _get_nc()
    pointer_input = np.ascontiguousarray(pointer_input, dtype=np.float32)
    W1 = np.ascontiguousarray(W1, dtype=np.float32)
    W2 = np.ascontiguousarray(W2, dtype=np.float32)
    in_maps = [
        {
            "p": pointer_input[i * B_LOC : (i + 1) * B_LOC],
            "w1": W1,
            "w2": W2,
        }
        for i in range(N_CORES)
    ]
    kw = dict(trace_kwargs or {})
    res = run_bass_kernel_spmd(
        nc, in_maps, list(range(N_CORES)), trace=trace, **kw
    )
    out = np.concatenate([res.results[i]["out"] for i in range(N_CORES)], axis=0)
    return out[None].astype(np.float32), res


def kernel(pointer_input, h_t, W1, W2):
    # h_t only shifts scores by a per-batch constant, which softmax cancels;
    # it does not affect the output.
    out, _ = run_sharded(pointer_input, W1, W2, trace=False)
    return out
